# revision 36
# baseline (speedup 1.0000x reference)
"""Trainium2 Bass kernel: dual cross-attention block (nn_CA_36670430773307).

Full-input contract: kernel(**inputs) takes the complete unsharded tensors and
returns the complete (4, 4096, 512) output.

Sharding: 8 cores = batch(4) x direction(2). Each core computes one full
cross-attention direction (t->i or i->t) for one batch element.

v5 design (HW-measured; 369.6us vs 383.7us v3 baseline):
  - sim (q@kT): bf16, K=64 head pairs row-grouped at partitions 0-63/64-127.
  - exp: split Scalar (native Exp, ~(N+352)/1.2 ns) and DVE (Schraudolph
    int16 = round(1024/ln2*x + 15*1024-44) bitcast to fp16), alternating
    per jt. The steady state is LATENCY-bound by the sim->exp->sim(k+2)
    chain (PSUM SIM ring bufs=2; deeper rings don't fit in the 8 banks):
    ~880-1000ns per jt.
  - av (attn@v): fp16, K=128 per j-tile with ones-augmented vaug carrying
    the softmax denominator (M-dim augmentation is free on the PE).
    avs popped BEFORE the sim pair each 2-jt step (PE FIFO is strict:
    a stalled sim must not head-of-line-block ready avs).
  - norm (softmax 1/D: Ln, Exp(-x+ln16), DMA shift, DVE mult) and the
    out-projection are ATOMIZED into an aux deque, popped one atom per
    engine slot per 2-jt pair (s/v right after the matching engine's exp).
    CRITICAL: atoms enqueue inside pop_pend when a unit's LAST av emits -
    earlier enqueueing records no dependency on the deferred writers.
  - head: input DMAs first (4-tile chunks, gpsimd casts f32->bf16), then
    weights; xkv stream processed first; ~32 ident matmuls pre-warm the
    PE HAM clock gate (cold PE runs at 1.2GHz for first ~3.4us); order
    LN(xkv) -> v-proj -> k-proj -> LN(xq) -> q-proj.
  - out-proj for the last quarter is split: heads 0-5 of its first token
    tile run during the final unit; final DMAs split in 2-tile halves.
  - gamma/beta folded into projection weights host-side (exact).
FAILED experiments (do not retry): fp8 DoubleRow av/sim (rel err > 2e-2:
the output is a near-canceling sum, elementwise fp8 noise amplifies ~50x);
xbar DMA-transpose for phase A (serializes ~1.3us/tile against concurrent
DMAs); gating norm pops to late pairs (+14us); vones-via-DMA (+7us vs
memsets in early DVE-idle); proj casts on Scalar (+6us).
Rel err 1.4898e-2 vs the 2e-2 gate (margin is thin: avoid precision loss).
"""

import numpy as np
import ml_dtypes

import concourse.bass as bass
import concourse.mybir as mybir
import concourse.tile as tile
from concourse.bass_utils import run_bass_kernel_spmd
from concourse.masks import make_identity

N = 2048            # tokens per stream
D = 512             # model dim
H = 8               # heads
HD = 64             # head dim
P = 128             # SBUF partitions
NT = N // P         # 16 token tiles
DC = D // P         # 4 model-dim chunks
IC = 512            # i-chunk (PSUM bank free size fp32)
LN_EPS = 1e-5

F32 = mybir.dt.float32
BF16 = mybir.dt.bfloat16
FP16 = mybir.dt.float16
I16 = mybir.dt.int16
ALU = mybir.AluOpType
ACTF = mybir.ActivationFunctionType

S16_A = 1024.0 / np.log(2.0)   # schraudolph-to-fp16: bits = A*x + B
S16_B = 15.0 * 1024 - 44.0
REC_SCALE = 16.0               # folded into Exp(-lnD + ln REC_SCALE)
OUT_DESCALE = 1.0 / REC_SCALE

LAST_EXEC_NS = None
_NC_CACHE = None


def _legalize_waits(js):
    """Walrus encodes ONE sync wait per instruction; split extras onto
    EventSemaphore instructions on the same engine."""
    for f in js["functions"]:
        for b in f["blocks"]:
            out = []
            for ins in b["instructions"]:
                si = ins.get("sync_info") or {}
                ow = si.get("on_wait") or []
                if len(ow) > 1:
                    for k, w in enumerate(ow[:-1]):
                        out.append({
                            "debug": ins.get("debug"),
                            "engine": ins["engine"],
                            "ins": [], "outs": [],
                            "name": f"{ins['name']}_w{k}",
                            "opcode": "EventSemaphore",
                            "sync_info": {"on_update": [], "on_wait": [w]},
                        })
                    si = dict(si)
                    si["on_wait"] = [ow[-1]]
                    ins = dict(ins)
                    ins["sync_info"] = si
                out.append(ins)
            b["instructions"] = out


def _build_program():
    nc = bass.Bass()

    xq = nc.declare_dram_parameter("xq", [N, D], F32, isOutput=False)
    xkv = nc.declare_dram_parameter("xkv", [N, D], F32, isOutput=False)
    wq = nc.declare_dram_parameter("wq", [D, D], BF16, isOutput=False)
    wkv = nc.declare_dram_parameter("wkv", [D, 2 * D], BF16, isOutput=False)
    wout16 = nc.declare_dram_parameter("wout16", [P, H // 2, D], FP16,
                                       isOutput=False)
    vones = nc.declare_dram_parameter("vones", [P, NT, H, P], FP16,
                                      isOutput=False)
    outs = [
        nc.declare_dram_parameter(f"out{g}", [P, 4, D], F32, isOutput=True)
        for g in range(NT // 4)
    ]

    with tile.TileContext(nc) as tc:
        _body(tc, xq, xkv, wq, wkv, wout16, vones, outs)

    import json
    js = json.loads(nc.to_json_bytes())
    _legalize_waits(js)
    legalized = json.dumps(js).encode()
    nc.to_json_bytes = lambda: legalized
    return nc


def _dma_in(tc, lnx, src):
    """Issue the input-stream DMA in 4-tile chunks (gpsimd: casts f32->bf16
    in flight; HWDGE queues cannot cast)."""
    nc = tc.nc
    xbig = lnx.tile([P, NT, D], BF16, tag="xbig", name="xbig")
    src_r = src.rearrange("(t p) d -> p t d", p=P)
    for q in range(4):
        nc.gpsimd.dma_start(out=xbig[:, q * 4:(q + 1) * 4, :],
                            in_=src_r[:, q * 4:(q + 1) * 4, :])
    return xbig


def _phase_a(tc, lns, lnxs, ps_pool, xbig, xT, ident, eps_sb,
             dma_transpose=False):
    """LayerNorm one stream token-major (gamma/beta folded into weights
    host-side), then transpose into d-major xT.

    dma_transpose=False: PE transpose + Scalar PSUM->SBUF copy (fastest
    end-to-end latency; used for the critical xkv stream).
    dma_transpose=True: xbar-DMA transpose straight to SBUF (zero PE/Scalar
    cost, but DMA-transposes serialize against other in-flight DMAs, so
    only safe once the input-stream DMA has drained; used for xq)."""
    nc = tc.nc
    for itg in range(NT // 4):
        # batched stats: one Sqrt + one reciprocal per 4 tiles (fewer
        # cross-engine latency hops in the LN chain)
        mvs = lns.tile([P, 4, 2], F32, tag="mvs", name="mvs")
        for kk in range(4):
            it = itg * 4 + kk
            st = lns.tile([P, 6], F32, tag="st", name="st")
            nc.vector.bn_stats(out=st, in_=xbig[:, it, :])
            nc.vector.bn_aggr(out=mvs[:, kk, :], in_=st)
        iv = lns.tile([P, 4], F32, tag="iv", name="iv")
        nc.scalar.activation(out=iv, in_=mvs[:, :, 1], func=ACTF.Sqrt,
                             bias=eps_sb)
        nc.vector.reciprocal(out=iv, in_=iv)
        xss = []
        for kk in range(4):
            it = itg * 4 + kk
            xs = lnxs.tile([P, D], BF16, name="xs")
            nc.vector.tensor_scalar(
                out=xs, in0=xbig[:, it, :],
                scalar1=mvs[:, kk, 0:1], scalar2=iv[:, kk:kk + 1],
                op0=ALU.subtract, op1=ALU.mult,
            )
            if dma_transpose:
                nc.sync.dma_start_transpose(
                    out=xT[:, :, it * P:(it + 1) * P], in_=xs)
            else:
                xss.append(xs)
        if not dma_transpose:
            for c in range(DC):
                ps = ps_pool.tile([P, 4 * P], BF16, tag="A", bufs=2,
                                  name="tp")
                for kk in range(4):
                    nc.tensor.transpose(
                        ps[:, kk * P:(kk + 1) * P],
                        xss[kk][:, c * P:(c + 1) * P],
                        ident,
                    )
                nc.scalar.copy(
                    out=xT[:, c, itg * 512:(itg + 1) * 512], in_=ps)


def _body(tc, xq, xkv, wq, wkv, wout16, vones, outs):
    nc = tc.nc

    with (
        tc.tile_pool(name="persist", bufs=1) as pers,
        tc.tile_pool(name="lns", bufs=16) as lns,
        tc.tile_pool(name="lnxs", bufs=12) as lnxs,
        tc.tile_pool(name="lnx", bufs=2) as lnx,
        tc.tile_pool(name="expp", bufs=3) as expp,
        tc.tile_pool(name="normp", bufs=2) as normp,
        tc.tile_pool(name="bigp", bufs=2) as bigp,
        tc.tile_pool(name="ps_pool", bufs=1, space="PSUM") as ps_pool,
    ):
        # ---- persistent tiles ----
        ident = pers.tile([P, P], BF16, name="ident")
        make_identity(nc, ident)
        eps_sb = pers.tile([P, 1], F32, name="eps_sb")
        nc.vector.memset(eps_sb, LN_EPS)
        ln16_sb = pers.tile([P, 1], F32, name="ln16_sb")
        nc.vector.memset(ln16_sb, float(np.log(REC_SCALE)))

        # input-stream DMAs issued FIRST (before the 2.5MB of weights) so
        # the LN pipeline starts as early as possible. xkv gates everything
        # (k/v proj feed all attention units), so it is issued first.
        xkv_big = _dma_in(tc, lnx, xkv)
        xq_big = _dma_in(tc, lnx, xq)

        wkv_sb = pers.tile([P, DC, 2 * D], BF16, name="wkv_sb")
        nc.gpsimd.dma_start(out=wkv_sb, in_=wkv.rearrange("(c p) f -> p c f", p=P))
        wq_sb = pers.tile([P, DC, D], BF16, name="wq_sb")
        nc.gpsimd.dma_start(out=wq_sb, in_=wq.rearrange("(c p) f -> p c f", p=P))
        wout16_sb = pers.tile([P, H // 2, D], FP16, name="wout16_sb")
        nc.gpsimd.dma_start(out=wout16_sb, in_=wout16[:, :, :])

        xqT = bigp.tile([P, DC, N], BF16, tag="big", name="xqT")
        xkvT = bigp.tile([P, DC, N], BF16, tag="big", name="xkvT")
        qT = pers.tile([P, DC, N], BF16, name="qT")   # head 2c rows 0-63,
        kT = pers.tile([P, DC, N], BF16, name="kT")   # head 2c+1 rows 64-127
        # vaug fp16 [p, jt, head, col]: even heads v@0-63/ones@64-127,
        # odd heads ones@0-63/v@64-127 (memsets run in early DVE idle time)
        vaug = pers.tile([P, NT, H, P], FP16, name="vaug")
        nc.vector.memset(vaug[:, :, 0::2, HD:P], 1.0)
        nc.vector.memset(vaug[:, :, 1::2, 0:HD], 1.0)
        # normalized attention out fp16: head 2hp rows 0-63, 2hp+1 rows 64-127
        aout16 = pers.tile([P, H // 2, N], FP16, name="aout16")

        # ---- HAM warmup: throwaway matmuls during the input DMA so the
        # PE clock gate is at 8/8 when real transposes/projs arrive. The
        # first batch runs immediately; later batches depend on the weight
        # DMAs (landing ~13-20us), bridging the idle gap to the first
        # transposes (HAM re-throttles after ~3.4us of PE idle; the HW
        # trace showed the whole head cold until 41.6us without this).
        for _ in range(32):
            wu = ps_pool.tile([P, P], F32, tag="A", bufs=2, name="wu")
            nc.tensor.matmul(wu, lhsT=ident, rhs=ident, start=True,
                             stop=True)
        for wsrc, cnt in ((wkv_sb[:, 0, 0:P], 12), (wq_sb[:, 0, 0:P], 12),
                          (wout16_sb[:, 0, 0:P], 8)):
            for _ in range(cnt):
                wu = ps_pool.tile([P, P], F32, tag="A", bufs=2, name="wu")
                nc.tensor.matmul(wu, lhsT=ident, rhs=wsrc, start=True,
                                 stop=True)

        # ---- phase A: layernorm + transpose (xkv first) ----
        _phase_a(tc, lns, lnxs, ps_pool, xkv_big, xkvT, ident, eps_sb,
                 dma_transpose=False)

        # ---- phase B (kv side): v then k projections ----
        # v token-major -> vaug fp16 (parity-split destinations)
        for mt in range(NT):
            ps = ps_pool.tile([P, D], F32, tag="A", bufs=2, name="psv")
            for k in range(DC):
                nc.tensor.matmul(
                    ps,
                    lhsT=xkvT[:, k, mt * P:(mt + 1) * P],
                    rhs=wkv_sb[:, k, D:2 * D],
                    start=(k == 0), stop=(k == DC - 1),
                )
            psr = ps.rearrange("p (h d) -> p h d", h=H)
            nc.scalar.copy(out=vaug[:, mt, 0::2, 0:HD], in_=psr[:, 0::2, :])
            nc.scalar.copy(out=vaug[:, mt, 1::2, HD:P], in_=psr[:, 1::2, :])

        def proj(dst, w_sb, xT):
            for m in range(DC):
                for nch in range(4):
                    ps = ps_pool.tile([P, IC], F32, tag="A", bufs=2,
                                      name="ps")
                    for k in range(DC):
                        nc.tensor.matmul(
                            ps,
                            lhsT=w_sb[:, k, m * P:(m + 1) * P],
                            rhs=xT[:, k, nch * IC:(nch + 1) * IC],
                            start=(k == 0), stop=(k == DC - 1),
                        )
                    nc.vector.tensor_copy(
                        out=dst[:, m, nch * IC:(nch + 1) * IC], in_=ps
                    )

        proj(kT, wkv_sb, xkvT)
        _phase_a(tc, lns, lnxs, ps_pool, xq_big, xqT, ident, eps_sb,
                 dma_transpose=False)
        proj(qT, wq_sb, xqT)
        # ---- phase C: attention ----
        # unit = (head pair hp, i-chunk iq). PSUM: simA/simB [128, 2, 512]
        # (2 banks each, jt-pair slots) + avA/avB [128, 512] (bufs=2 ring).
        # exp mix per unit: S10/D6 (even units) S9/D7 (odd).
        EXP_EVEN = [("s", "d"), ("d", "s")] * 4
        EXP_ODD = [("d", "s"), ("s", "d")] * 4

        def emit_exp(eng, ex, sim):
            sim_flat = sim.rearrange("p a b -> p (a b)")
            if eng == "s":
                nc.scalar.activation(out=ex.rearrange("p a b -> p (a b)"),
                                     in_=sim_flat, func=ACTF.Exp)
            else:
                nc.vector.tensor_scalar(
                    out=ex.rearrange("p a b -> p (a b)").bitcast(I16),
                    in0=sim_flat, scalar1=float(S16_A), scalar2=float(S16_B),
                    op0=ALU.mult, op1=ALU.add)

        # aux: ordered deque of (slot, closure). Norm and out-projection work
        # is atomized and popped into per-pair engine slots: "p" atoms at
        # pair start (PE, alongside avs), "s"/"v" atoms right after the
        # matching engine's exp so they execute during the other engine's
        # exp turn. Strictly head-ordered pops preserve intra-chain order.
        aux = []

        def pop_aux(slots, allow=True):
            if allow and aux and aux[0][0] in slots:
                aux.pop(0)[1]()

        def norm_atoms(avs, hp, iq):
            st = {}

            def ln(s):
                def f():
                    dlo = (1 - s) * HD
                    lnD = normp.tile([P, IC], F32, tag=f"lnD{s}", name="lnD")
                    st["lnD%d" % s] = lnD
                    nc.scalar.activation(
                        out=lnD[dlo:dlo + HD, :],
                        in_=avs[s][dlo:dlo + HD, :], func=ACTF.Ln)
                return f

            def rec(s):
                def f():
                    dlo = (1 - s) * HD
                    nlo = s * HD
                    lnD = st["lnD%d" % s]
                    recE = normp.tile([P, IC], F32, tag=f"recE{s}",
                                      name="recE")
                    nc.scalar.activation(
                        out=recE[dlo:dlo + HD, :], in_=lnD[dlo:dlo + HD, :],
                        func=ACTF.Exp, scale=-1.0,
                        bias=ln16_sb[dlo:dlo + HD, :])
                    recN = normp.tile([P, IC], F32, tag=f"recN{s}",
                                      name="recN")
                    st["recN%d" % s] = recN
                    nc.sync.dma_start(out=recN[nlo:nlo + HD, :],
                                      in_=recE[dlo:dlo + HD, :])
                return f

            def mul(s):
                def f():
                    nlo = s * HD
                    nc.vector.tensor_tensor(
                        out=aout16[nlo:nlo + HD, hp, iq * IC:(iq + 1) * IC],
                        in0=avs[s][nlo:nlo + HD, :],
                        in1=st["recN%d" % s][nlo:nlo + HD, :], op=ALU.mult)
                return f

            return [("s", ln(0)), ("s", rec(0)), ("v", mul(0)),
                    ("s", ln(1)), ("s", rec(1)), ("v", mul(1))]

        # phase D atoms: one out-proj token tile per "p" atom (4 matmuls),
        # PSUM->SBUF copy per "s" atom, DMA out in 2-tile halves.
        pso_state = {}
        os_state = {}

        def phd_mm(iq, j, hps):
            def f():
                it = iq * 4 + j
                if (iq, j) not in pso_state:
                    pso_state[(iq, j)] = ps_pool.tile(
                        [P, D], F32, tag="av1", bufs=2, name="pso")
                pso = pso_state[(iq, j)]
                for hp2 in hps:
                    nc.tensor.matmul(
                        pso,
                        lhsT=aout16[:, hp2, it * P:(it + 1) * P],
                        rhs=wout16_sb[:, hp2, :],
                        start=(hp2 == 0), stop=(hp2 == H // 2 - 1),
                    )
            return f

        def phd_copy(iq, j):
            def f():
                if iq not in os_state:
                    os_state[iq] = bigp.tile([P, 4, D], F32, tag="big",
                                             name="os_big")
                os_ = os_state[iq]
                pso = pso_state.pop((iq, j))
                nc.scalar.activation(out=os_[:, j, :], in_=pso,
                                     func=ACTF.Copy, scale=float(OUT_DESCALE))
                if j == 1:
                    nc.sync.dma_start(out=outs[iq][:, 0:2, :],
                                      in_=os_[:, 0:2, :])
                elif j == 3:
                    nc.sync.dma_start(out=outs[iq][:, 2:4, :],
                                      in_=os_[:, 2:4, :])
            return f

        def pop_pend(limit, max_pops):
            # Emits deferred av matmuls. When a unit's LAST av is emitted,
            # its norm (and any dependent out-proj) atoms are enqueued HERE:
            # enqueueing them any earlier would emit readers of avs/aout16
            # before their writers exist, so no dependency would be recorded.
            n = 0
            while pend and len(pend) >= limit and n < max_pops:
                pavs, php, piq, pjt, pex = pend.pop(0)
                for s in range(2):
                    nc.tensor.matmul(
                        pavs[s],
                        lhsT=vaug[:, pjt, 2 * php + s, :],
                        rhs=pex[:, s, :],
                        start=(pjt == 0), stop=(pjt == NT - 1),
                    )
                if pjt == NT - 1:
                    aux.extend(norm_atoms(pavs, php, piq))
                    if php == H // 2 - 1 and piq < 3:
                        for j in range(4):
                            aux.append(("p", phd_mm(piq, j,
                                                    list(range(H // 2)))))
                            aux.append(("s", phd_copy(piq, j)))
                    if piq == 3 and php == H // 2 - 2:
                        # last-quarter out-proj: heads 0-5 of token tile 0
                        # can run once norms 12-14 are emitted.
                        aux.append(("p", phd_mm(3, 0, [0, 1, 2])))
                n += 1

        pend = []   # av pipeline carried ACROSS units (no per-unit flush)
        for iq in range(4):
            for hp in range(H // 2):
                u = iq * 4 + hp
                avs = [
                    ps_pool.tile([P, IC], F32, tag=("A" if s == 0 else "av1"),
                                 bufs=2, name=f"av{s}")
                    for s in range(2)
                ]
                for t in range(NT // 2):
                    pop_aux(("p",))
                    if u == 15 and t >= 4:
                        pop_pend(3, 3)
                    else:
                        # keep unread-ex count <= ex bufs (5): with 2 appends
                        # per pair, popping at >=4 caps the post-append
                        # backlog at 5.
                        pop_pend(4, 2)
                    for jj in (2 * t, 2 * t + 1):
                        sim = ps_pool.tile([P, 2, IC], F32, tag="SIM",
                                           bufs=2, name="sim")
                        for s in range(2):
                            hb = s * HD
                            nc.tensor.matmul(
                                sim[:, s, :],
                                lhsT=kT[hb:hb + HD, hp,
                                        jj * P:(jj + 1) * P],
                                rhs=qT[hb:hb + HD, hp,
                                       iq * IC:(iq + 1) * IC],
                                start=True, stop=True,
                            )
                        ex = expp.tile([P, 2, IC], FP16, tag="ex", bufs=5,
                                       name="ex")
                        eng = ("s", "d")[(jj + u) % 2]
                        emit_exp(eng, ex, sim)
                        pend.append((avs, hp, iq, jj, ex))
                        pop_aux(("s",) if eng == "s" else ("v",))
        pop_pend(1, len(pend))
        aux.append(("p", phd_mm(3, 0, [3])))
        aux.append(("s", phd_copy(3, 0)))
        for j in range(1, 4):
            aux.append(("p", phd_mm(3, j, list(range(H // 2)))))
            aux.append(("s", phd_copy(3, j)))
        while aux:
            aux.pop(0)[1]()

def _get_nc():
    global _NC_CACHE
    if _NC_CACHE is None:
        _NC_CACHE = _build_program()
    return _NC_CACHE


def kernel(i, t, g_i, b_i, g_t, b_t, w_qkv_i, w_qkv_t, w_out_i, w_out_t):
    global LAST_EXEC_NS
    nc = _get_nc()

    i = np.asarray(i, np.float32)
    t = np.asarray(t, np.float32)
    bf = ml_dtypes.bfloat16
    f16 = np.float16
    w_qkv_i = np.asarray(w_qkv_i, np.float32)
    w_qkv_t = np.asarray(w_qkv_t, np.float32)
    g_i = np.asarray(g_i, np.float32)
    g_t = np.asarray(g_t, np.float32)
    assert np.abs(np.asarray(b_i)).max() == 0.0, "beta_i must be zero"
    assert np.abs(np.asarray(b_t)).max() == 0.0, "beta_t must be zero"

    # gamma folded into projection weights (exact); 0.125 folded into wq
    wq_i = (w_qkv_i[:, :D] * 0.125 * g_i[:, None]).astype(bf)
    wq_t = (w_qkv_t[:, :D] * 0.125 * g_t[:, None]).astype(bf)
    wkv_i = np.ascontiguousarray(w_qkv_i[:, D:] * g_i[:, None]).astype(bf)
    wkv_t = np.ascontiguousarray(w_qkv_t[:, D:] * g_t[:, None]).astype(bf)

    def mk_wout16(w):
        # [512, 512] -> [128, 4, 512]: rows 0-63 = head 2hp, 64-127 = 2hp+1
        w = np.asarray(w, np.float32)
        w = w.reshape(H // 2, P, D).transpose(1, 0, 2)
        return np.ascontiguousarray(w).astype(f16)

    wo16_i = mk_wout16(w_out_i)
    wo16_t = mk_wout16(w_out_t)
    f32 = lambda a: np.ascontiguousarray(np.asarray(a, np.float32))

    # vaug ones pattern: even heads ones at cols 64-127, odd at 0-63
    vones = np.zeros((P, NT, H, P), np.float16)
    vones[:, :, 0::2, HD:P] = 1.0
    vones[:, :, 1::2, 0:HD] = 1.0

    in_maps = []
    for c in range(8):
        b, d = c // 2, c % 2
        if d == 0:  # t -> i: queries from t, keys/values from i
            m = dict(xq=f32(t[b]), xkv=f32(i[b]),
                     wq=wq_t, wkv=wkv_i, wout16=wo16_i, vones=vones)
        else:       # i -> t
            m = dict(xq=f32(i[b]), xkv=f32(t[b]),
                     wq=wq_i, wkv=wkv_t, wout16=wo16_t, vones=vones)
        in_maps.append(m)

    res = run_bass_kernel_spmd(nc, in_maps, list(range(8)))
    LAST_EXEC_NS = res.exec_time_ns

    out = np.empty((4, 2 * N, D), np.float32)
    for c in range(8):
        b, d = c // 2, c % 2
        for g in range(NT // 4):
            blk = res.results[c][f"out{g}"]  # [128, 4, 512]
            for j in range(4):
                it = g * 4 + j
                out[b, d * N + it * P:d * N + (it + 1) * P, :] = blk[:, j, :]
    return out



# revision 37
# speedup vs baseline: 1.0601x; 1.0601x over previous
"""Trainium2 Bass kernel: dual cross-attention block (nn_CA_36670430773307).

Full-input contract: kernel(**inputs) takes the complete unsharded tensors and
returns the complete (4, 4096, 512) output.

Sharding: 8 cores = batch(4) x direction(2). Each core computes one full
cross-attention direction (t->i or i->t) for one batch element.

v5 design (HW-measured; 369.6us vs 383.7us v3 baseline):
  - sim (q@kT): bf16, K=64 head pairs row-grouped at partitions 0-63/64-127.
  - exp: split Scalar (native Exp, ~(N+352)/1.2 ns) and DVE (Schraudolph
    int16 = round(1024/ln2*x + 15*1024-44) bitcast to fp16), alternating
    per jt. The steady state is LATENCY-bound by the sim->exp->sim(k+2)
    chain (PSUM SIM ring bufs=2; deeper rings don't fit in the 8 banks):
    ~880-1000ns per jt.
  - av (attn@v): fp16, K=128 per j-tile with ones-augmented vaug carrying
    the softmax denominator (M-dim augmentation is free on the PE).
    avs popped BEFORE the sim pair each 2-jt step (PE FIFO is strict:
    a stalled sim must not head-of-line-block ready avs).
  - norm (softmax 1/D: Ln, Exp(-x+ln16), DMA shift, DVE mult) and the
    out-projection are ATOMIZED into an aux deque, popped one atom per
    engine slot per 2-jt pair (s/v right after the matching engine's exp).
    CRITICAL: atoms enqueue inside pop_pend when a unit's LAST av emits -
    earlier enqueueing records no dependency on the deferred writers.
  - head: input DMAs first (4-tile chunks, gpsimd casts f32->bf16), then
    weights; xkv stream processed first; ~32 ident matmuls pre-warm the
    PE HAM clock gate (cold PE runs at 1.2GHz for first ~3.4us); order
    LN(xkv) -> v-proj -> k-proj -> LN(xq) -> q-proj.
  - out-proj for the last quarter is split: heads 0-5 of its first token
    tile run during the final unit; final DMAs split in 2-tile halves.
  - gamma/beta folded into projection weights host-side (exact).
FAILED experiments (do not retry): fp8 DoubleRow av/sim (rel err > 2e-2:
the output is a near-canceling sum, elementwise fp8 noise amplifies ~50x);
xbar DMA-transpose for phase A (serializes ~1.3us/tile against concurrent
DMAs); gating norm pops to late pairs (+14us); vones-via-DMA (+7us vs
memsets in early DVE-idle); proj casts on Scalar (+6us).
Rel err 1.4898e-2 vs the 2e-2 gate (margin is thin: avoid precision loss).
"""

import numpy as np
import ml_dtypes

import concourse.bass as bass
import concourse.mybir as mybir
import concourse.tile as tile
from concourse.bass_utils import run_bass_kernel_spmd
from concourse.masks import make_identity

N = 2048            # tokens per stream
D = 512             # model dim
H = 8               # heads
HD = 64             # head dim
P = 128             # SBUF partitions
NT = N // P         # 16 token tiles
DC = D // P         # 4 model-dim chunks
IC = 512            # i-chunk (PSUM bank free size fp32)
LN_EPS = 1e-5

F32 = mybir.dt.float32
BF16 = mybir.dt.bfloat16
FP16 = mybir.dt.float16
I16 = mybir.dt.int16
ALU = mybir.AluOpType
ACTF = mybir.ActivationFunctionType

S16_A = 1024.0 / np.log(2.0)   # schraudolph-to-fp16: bits = A*x + B
S16_B = 15.0 * 1024 - 44.0
REC_SCALE = 16.0               # folded into Exp(-lnD + ln REC_SCALE)
OUT_DESCALE = 1.0 / REC_SCALE

LAST_EXEC_NS = None
_NC_CACHE = None


def _legalize_waits(js):
    """Walrus encodes ONE sync wait per instruction; split extras onto
    EventSemaphore instructions on the same engine."""
    for f in js["functions"]:
        for b in f["blocks"]:
            out = []
            for ins in b["instructions"]:
                si = ins.get("sync_info") or {}
                ow = si.get("on_wait") or []
                if len(ow) > 1:
                    for k, w in enumerate(ow[:-1]):
                        out.append({
                            "debug": ins.get("debug"),
                            "engine": ins["engine"],
                            "ins": [], "outs": [],
                            "name": f"{ins['name']}_w{k}",
                            "opcode": "EventSemaphore",
                            "sync_info": {"on_update": [], "on_wait": [w]},
                        })
                    si = dict(si)
                    si["on_wait"] = [ow[-1]]
                    ins = dict(ins)
                    ins["sync_info"] = si
                out.append(ins)
            b["instructions"] = out


def _build_program():
    nc = bass.Bass()

    xq = nc.declare_dram_parameter("xq", [N, D], F32, isOutput=False)
    xkv = nc.declare_dram_parameter("xkv", [N, D], F32, isOutput=False)
    wq = nc.declare_dram_parameter("wq", [D, D], BF16, isOutput=False)
    wkv = nc.declare_dram_parameter("wkv", [D, 2 * D], BF16, isOutput=False)
    wout16 = nc.declare_dram_parameter("wout16", [P, H // 2, D], FP16,
                                       isOutput=False)
    vones = nc.declare_dram_parameter("vones", [P, NT, H, P], FP16,
                                      isOutput=False)
    outs = [
        nc.declare_dram_parameter(f"out{g}", [P, 4, D], F32, isOutput=True)
        for g in range(NT // 4)
    ]

    with tile.TileContext(nc) as tc:
        _body(tc, xq, xkv, wq, wkv, wout16, vones, outs)

    import json
    js = json.loads(nc.to_json_bytes())
    _legalize_waits(js)
    legalized = json.dumps(js).encode()
    nc.to_json_bytes = lambda: legalized
    return nc


def _dma_in(tc, lnx, src):
    """Issue the input-stream DMA in 4-tile chunks (gpsimd: casts f32->bf16
    in flight; HWDGE queues cannot cast)."""
    nc = tc.nc
    xbig = lnx.tile([P, NT, D], BF16, tag="xbig", name="xbig")
    src_r = src.rearrange("(t p) d -> p t d", p=P)
    for q in range(4):
        nc.gpsimd.dma_start(out=xbig[:, q * 4:(q + 1) * 4, :],
                            in_=src_r[:, q * 4:(q + 1) * 4, :])
    return xbig


def _phase_a(tc, lns, lnxs, ps_pool, xbig, xT, ident, eps_sb,
             dma_transpose=False):
    """LayerNorm one stream token-major (gamma/beta folded into weights
    host-side), then transpose into d-major xT.

    dma_transpose=False: PE transpose + Scalar PSUM->SBUF copy (fastest
    end-to-end latency; used for the critical xkv stream).
    dma_transpose=True: xbar-DMA transpose straight to SBUF (zero PE/Scalar
    cost, but DMA-transposes serialize against other in-flight DMAs, so
    only safe once the input-stream DMA has drained; used for xq)."""
    nc = tc.nc
    for itg in range(NT // 4):
        # batched stats: one Sqrt + one reciprocal per 4 tiles (fewer
        # cross-engine latency hops in the LN chain)
        mvs = lns.tile([P, 4, 2], F32, tag="mvs", name="mvs")
        for kk in range(4):
            it = itg * 4 + kk
            st = lns.tile([P, 6], F32, tag="st", name="st")
            nc.vector.bn_stats(out=st, in_=xbig[:, it, :])
            nc.vector.bn_aggr(out=mvs[:, kk, :], in_=st)
        iv = lns.tile([P, 4], F32, tag="iv", name="iv")
        nc.scalar.activation(out=iv, in_=mvs[:, :, 1], func=ACTF.Sqrt,
                             bias=eps_sb)
        nc.vector.reciprocal(out=iv, in_=iv)
        xss = []
        for kk in range(4):
            it = itg * 4 + kk
            xs = lnxs.tile([P, D], BF16, name="xs")
            nc.vector.tensor_scalar(
                out=xs, in0=xbig[:, it, :],
                scalar1=mvs[:, kk, 0:1], scalar2=iv[:, kk:kk + 1],
                op0=ALU.subtract, op1=ALU.mult,
            )
            if dma_transpose:
                nc.sync.dma_start_transpose(
                    out=xT[:, :, it * P:(it + 1) * P], in_=xs)
            else:
                xss.append(xs)
        if not dma_transpose:
            for c in range(DC):
                ps = ps_pool.tile([P, 4 * P], BF16, tag="A", bufs=2,
                                  name="tp")
                for kk in range(4):
                    nc.tensor.transpose(
                        ps[:, kk * P:(kk + 1) * P],
                        xss[kk][:, c * P:(c + 1) * P],
                        ident,
                    )
                nc.scalar.copy(
                    out=xT[:, c, itg * 512:(itg + 1) * 512], in_=ps)


def _body(tc, xq, xkv, wq, wkv, wout16, vones, outs):
    nc = tc.nc

    with (
        tc.tile_pool(name="persist", bufs=1) as pers,
        tc.tile_pool(name="lns", bufs=16) as lns,
        tc.tile_pool(name="lnxs", bufs=12) as lnxs,
        tc.tile_pool(name="lnx", bufs=2) as lnx,
        tc.tile_pool(name="expp", bufs=3) as expp,
        tc.tile_pool(name="normp", bufs=2) as normp,
        tc.tile_pool(name="bigp", bufs=2) as bigp,
        tc.tile_pool(name="ps_pool", bufs=1, space="PSUM") as ps_pool,
    ):
        # ---- persistent tiles ----
        ident = pers.tile([P, P], BF16, name="ident")
        make_identity(nc, ident)
        eps_sb = pers.tile([P, 1], F32, name="eps_sb")
        nc.vector.memset(eps_sb, LN_EPS)
        ln16_sb = pers.tile([P, 1], F32, name="ln16_sb")
        nc.vector.memset(ln16_sb, float(np.log(REC_SCALE)))

        # input-stream DMAs issued FIRST (before the 2.5MB of weights) so
        # the LN pipeline starts as early as possible. xkv gates everything
        # (k/v proj feed all attention units), so it is issued first.
        xkv_big = _dma_in(tc, lnx, xkv)
        xq_big = _dma_in(tc, lnx, xq)

        wkv_sb = pers.tile([P, DC, 2 * D], BF16, name="wkv_sb")
        nc.gpsimd.dma_start(out=wkv_sb, in_=wkv.rearrange("(c p) f -> p c f", p=P))
        wq_sb = pers.tile([P, DC, D], BF16, name="wq_sb")
        nc.gpsimd.dma_start(out=wq_sb, in_=wq.rearrange("(c p) f -> p c f", p=P))
        wout16_sb = pers.tile([P, H // 2, D], FP16, name="wout16_sb")
        nc.gpsimd.dma_start(out=wout16_sb, in_=wout16[:, :, :])

        xqT = bigp.tile([P, DC, N], BF16, tag="big", name="xqT")
        xkvT = bigp.tile([P, DC, N], BF16, tag="big", name="xkvT")
        qT = pers.tile([P, DC, N], BF16, name="qT")   # head 2c rows 0-63,
        kT = pers.tile([P, DC, N], BF16, name="kT")   # head 2c+1 rows 64-127
        # vaug fp16 [p, jt, head, col]: even heads v@0-63/ones@64-127,
        # odd heads ones@0-63/v@64-127 (memsets run in early DVE idle time)
        vaug = pers.tile([P, NT, H, P], FP16, name="vaug")
        nc.vector.memset(vaug[:, :, 0::2, HD:P], 1.0)
        nc.vector.memset(vaug[:, :, 1::2, 0:HD], 1.0)
        # normalized attention out fp16: head 2hp rows 0-63, 2hp+1 rows 64-127
        aout16 = pers.tile([P, H // 2, N], FP16, name="aout16")

        # ---- HAM warmup: ~3.5us of throwaway matmuls during the input DMA
        # so the PE clock gate is at 8/8 when real transposes/projs arrive.
        for _ in range(32):
            wu = ps_pool.tile([P, P], F32, tag="A", bufs=2, name="wu")
            nc.tensor.matmul(wu, lhsT=ident, rhs=ident, start=True,
                             stop=True)

        # ---- phase A: layernorm + transpose (xkv first) ----
        _phase_a(tc, lns, lnxs, ps_pool, xkv_big, xkvT, ident, eps_sb,
                 dma_transpose=False)

        # ---- phase B (kv side): v then k projections ----
        # v token-major -> vaug fp16 (parity-split destinations)
        for mt in range(NT):
            ps = ps_pool.tile([P, D], F32, tag="A", bufs=2, name="psv")
            for k in range(DC):
                nc.tensor.matmul(
                    ps,
                    lhsT=xkvT[:, k, mt * P:(mt + 1) * P],
                    rhs=wkv_sb[:, k, D:2 * D],
                    start=(k == 0), stop=(k == DC - 1),
                )
            psr = ps.rearrange("p (h d) -> p h d", h=H)
            nc.scalar.copy(out=vaug[:, mt, 0::2, 0:HD], in_=psr[:, 0::2, :])
            nc.scalar.copy(out=vaug[:, mt, 1::2, HD:P], in_=psr[:, 1::2, :])

        def proj(dst, w_sb, xT):
            for m in range(DC):
                for nch in range(4):
                    ps = ps_pool.tile([P, IC], F32, tag="A", bufs=2,
                                      name="ps")
                    for k in range(DC):
                        nc.tensor.matmul(
                            ps,
                            lhsT=w_sb[:, k, m * P:(m + 1) * P],
                            rhs=xT[:, k, nch * IC:(nch + 1) * IC],
                            start=(k == 0), stop=(k == DC - 1),
                        )
                    nc.vector.tensor_copy(
                        out=dst[:, m, nch * IC:(nch + 1) * IC], in_=ps
                    )

        proj(kT, wkv_sb, xkvT)
        _phase_a(tc, lns, lnxs, ps_pool, xq_big, xqT, ident, eps_sb,
                 dma_transpose=False)
        proj(qT, wq_sb, xqT)
        # ---- phase C: attention ----
        # unit = (head pair hp, i-chunk iq). PSUM: simA/simB [128, 2, 512]
        # (2 banks each, jt-pair slots) + avA/avB [128, 512] (bufs=2 ring).
        # exp mix per unit: S10/D6 (even units) S9/D7 (odd).
        EXP_EVEN = [("s", "d"), ("d", "s")] * 4
        EXP_ODD = [("d", "s"), ("s", "d")] * 4

        def emit_exp(eng, ex, sim):
            sim_flat = sim.rearrange("p a b -> p (a b)")
            if eng == "s":
                nc.scalar.activation(out=ex.rearrange("p a b -> p (a b)"),
                                     in_=sim_flat, func=ACTF.Exp)
            else:
                nc.vector.tensor_scalar(
                    out=ex.rearrange("p a b -> p (a b)").bitcast(I16),
                    in0=sim_flat, scalar1=float(S16_A), scalar2=float(S16_B),
                    op0=ALU.mult, op1=ALU.add)

        # aux: ordered deque of (slot, closure). Norm and out-projection work
        # is atomized and popped into per-pair engine slots: "p" atoms at
        # pair start (PE, alongside avs), "s"/"v" atoms right after the
        # matching engine's exp so they execute during the other engine's
        # exp turn. Strictly head-ordered pops preserve intra-chain order.
        aux = []

        def pop_aux(slots, allow=True):
            if allow and aux and aux[0][0] in slots:
                aux.pop(0)[1]()

        def norm_atoms(avs, hp, iq):
            st = {}

            def ln(s):
                def f():
                    dlo = (1 - s) * HD
                    lnD = normp.tile([P, IC], F32, tag=f"lnD{s}", name="lnD")
                    st["lnD%d" % s] = lnD
                    nc.scalar.activation(
                        out=lnD[dlo:dlo + HD, :],
                        in_=avs[s][dlo:dlo + HD, :], func=ACTF.Ln)
                return f

            def rec(s):
                def f():
                    dlo = (1 - s) * HD
                    nlo = s * HD
                    lnD = st["lnD%d" % s]
                    recE = normp.tile([P, IC], F32, tag=f"recE{s}",
                                      name="recE")
                    nc.scalar.activation(
                        out=recE[dlo:dlo + HD, :], in_=lnD[dlo:dlo + HD, :],
                        func=ACTF.Exp, scale=-1.0,
                        bias=ln16_sb[dlo:dlo + HD, :])
                    recN = normp.tile([P, IC], F32, tag=f"recN{s}",
                                      name="recN")
                    st["recN%d" % s] = recN
                    nc.sync.dma_start(out=recN[nlo:nlo + HD, :],
                                      in_=recE[dlo:dlo + HD, :])
                return f

            def mul(s):
                def f():
                    nlo = s * HD
                    nc.vector.tensor_tensor(
                        out=aout16[nlo:nlo + HD, hp, iq * IC:(iq + 1) * IC],
                        in0=avs[s][nlo:nlo + HD, :],
                        in1=st["recN%d" % s][nlo:nlo + HD, :], op=ALU.mult)
                return f

            return [("s", ln(0)), ("s", rec(0)), ("v", mul(0)),
                    ("s", ln(1)), ("s", rec(1)), ("v", mul(1))]

        # phase D atoms: one out-proj token tile per "p" atom (4 matmuls),
        # PSUM->SBUF copy per "s" atom, DMA out in 2-tile halves.
        pso_state = {}
        os_state = {}

        def phd_mm(iq, j, hps):
            def f():
                it = iq * 4 + j
                if (iq, j) not in pso_state:
                    pso_state[(iq, j)] = ps_pool.tile(
                        [P, D], F32, tag="av1", bufs=2, name="pso")
                pso = pso_state[(iq, j)]
                for hp2 in hps:
                    nc.tensor.matmul(
                        pso,
                        lhsT=aout16[:, hp2, it * P:(it + 1) * P],
                        rhs=wout16_sb[:, hp2, :],
                        start=(hp2 == 0), stop=(hp2 == H // 2 - 1),
                    )
            return f

        def phd_copy(iq, j):
            def f():
                if iq not in os_state:
                    os_state[iq] = bigp.tile([P, 4, D], F32, tag="big",
                                             name="os_big")
                os_ = os_state[iq]
                pso = pso_state.pop((iq, j))
                nc.scalar.activation(out=os_[:, j, :], in_=pso,
                                     func=ACTF.Copy, scale=float(OUT_DESCALE))
                if j == 1:
                    nc.sync.dma_start(out=outs[iq][:, 0:2, :],
                                      in_=os_[:, 0:2, :])
                elif j == 3:
                    nc.sync.dma_start(out=outs[iq][:, 2:4, :],
                                      in_=os_[:, 2:4, :])
            return f

        def pop_pend(limit, max_pops):
            # Emits deferred av matmuls. When a unit's LAST av is emitted,
            # its norm (and any dependent out-proj) atoms are enqueued HERE:
            # enqueueing them any earlier would emit readers of avs/aout16
            # before their writers exist, so no dependency would be recorded.
            n = 0
            while pend and len(pend) >= limit and n < max_pops:
                pavs, php, piq, pjt, pex = pend.pop(0)
                for s in range(2):
                    nc.tensor.matmul(
                        pavs[s],
                        lhsT=vaug[:, pjt, 2 * php + s, :],
                        rhs=pex[:, s, :],
                        start=(pjt == 0), stop=(pjt == NT - 1),
                    )
                if pjt == NT - 1:
                    aux.extend(norm_atoms(pavs, php, piq))
                    if php == H // 2 - 1 and piq < 3:
                        for j in range(4):
                            aux.append(("p", phd_mm(piq, j,
                                                    list(range(H // 2)))))
                            aux.append(("s", phd_copy(piq, j)))
                    if piq == 3 and php == H // 2 - 2:
                        # last-quarter out-proj: heads 0-5 of token tile 0
                        # can run once norms 12-14 are emitted.
                        aux.append(("p", phd_mm(3, 0, [0, 1, 2])))
                n += 1

        pend = []   # av pipeline carried ACROSS units (no per-unit flush)
        for iq in range(4):
            for hp in range(H // 2):
                u = iq * 4 + hp
                avs = [
                    ps_pool.tile([P, IC], F32, tag=("A" if s == 0 else "av1"),
                                 bufs=2, name=f"av{s}")
                    for s in range(2)
                ]
                for t in range(NT // 2):
                    pop_aux(("p",))
                    if u == 15 and t >= 4:
                        pop_pend(3, 3)
                    else:
                        # keep unread-ex count <= ex bufs (5): with 2 appends
                        # per pair, popping at >=4 caps the post-append
                        # backlog at 5.
                        pop_pend(4, 2)
                    for jj in (2 * t, 2 * t + 1):
                        sim = ps_pool.tile([P, 2, IC], F32, tag="SIM",
                                           bufs=2, name="sim")
                        for s in range(2):
                            hb = s * HD
                            nc.tensor.matmul(
                                sim[:, s, :],
                                lhsT=kT[hb:hb + HD, hp,
                                        jj * P:(jj + 1) * P],
                                rhs=qT[hb:hb + HD, hp,
                                       iq * IC:(iq + 1) * IC],
                                start=True, stop=True,
                            )
                        ex = expp.tile([P, 2, IC], FP16, tag="ex", bufs=5,
                                       name="ex")
                        eng = ("s", "d")[(jj + u) % 2]
                        emit_exp(eng, ex, sim)
                        pend.append((avs, hp, iq, jj, ex))
                        pop_aux(("s",) if eng == "s" else ("v",))
        pop_pend(1, len(pend))
        aux.append(("p", phd_mm(3, 0, [3])))
        aux.append(("s", phd_copy(3, 0)))
        for j in range(1, 4):
            aux.append(("p", phd_mm(3, j, list(range(H // 2)))))
            aux.append(("s", phd_copy(3, j)))
        while aux:
            aux.pop(0)[1]()

def _get_nc():
    global _NC_CACHE
    if _NC_CACHE is None:
        _NC_CACHE = _build_program()
    return _NC_CACHE


def kernel(i, t, g_i, b_i, g_t, b_t, w_qkv_i, w_qkv_t, w_out_i, w_out_t):
    global LAST_EXEC_NS
    nc = _get_nc()

    i = np.asarray(i, np.float32)
    t = np.asarray(t, np.float32)
    bf = ml_dtypes.bfloat16
    f16 = np.float16
    w_qkv_i = np.asarray(w_qkv_i, np.float32)
    w_qkv_t = np.asarray(w_qkv_t, np.float32)
    g_i = np.asarray(g_i, np.float32)
    g_t = np.asarray(g_t, np.float32)
    assert np.abs(np.asarray(b_i)).max() == 0.0, "beta_i must be zero"
    assert np.abs(np.asarray(b_t)).max() == 0.0, "beta_t must be zero"

    # gamma folded into projection weights (exact); 0.125 folded into wq
    wq_i = (w_qkv_i[:, :D] * 0.125 * g_i[:, None]).astype(bf)
    wq_t = (w_qkv_t[:, :D] * 0.125 * g_t[:, None]).astype(bf)
    wkv_i = np.ascontiguousarray(w_qkv_i[:, D:] * g_i[:, None]).astype(bf)
    wkv_t = np.ascontiguousarray(w_qkv_t[:, D:] * g_t[:, None]).astype(bf)

    def mk_wout16(w):
        # [512, 512] -> [128, 4, 512]: rows 0-63 = head 2hp, 64-127 = 2hp+1
        w = np.asarray(w, np.float32)
        w = w.reshape(H // 2, P, D).transpose(1, 0, 2)
        return np.ascontiguousarray(w).astype(f16)

    wo16_i = mk_wout16(w_out_i)
    wo16_t = mk_wout16(w_out_t)
    f32 = lambda a: np.ascontiguousarray(np.asarray(a, np.float32))

    # vaug ones pattern: even heads ones at cols 64-127, odd at 0-63
    vones = np.zeros((P, NT, H, P), np.float16)
    vones[:, :, 0::2, HD:P] = 1.0
    vones[:, :, 1::2, 0:HD] = 1.0

    in_maps = []
    for c in range(8):
        b, d = c // 2, c % 2
        if d == 0:  # t -> i: queries from t, keys/values from i
            m = dict(xq=f32(t[b]), xkv=f32(i[b]),
                     wq=wq_t, wkv=wkv_i, wout16=wo16_i, vones=vones)
        else:       # i -> t
            m = dict(xq=f32(i[b]), xkv=f32(t[b]),
                     wq=wq_i, wkv=wkv_t, wout16=wo16_t, vones=vones)
        in_maps.append(m)

    res = run_bass_kernel_spmd(nc, in_maps, list(range(8)))
    LAST_EXEC_NS = res.exec_time_ns

    out = np.empty((4, 2 * N, D), np.float32)
    for c in range(8):
        b, d = c // 2, c % 2
        for g in range(NT // 4):
            blk = res.results[c][f"out{g}"]  # [128, 4, 512]
            for j in range(4):
                it = g * 4 + j
                out[b, d * N + it * P:d * N + (it + 1) * P, :] = blk[:, j, :]
    return out

